# revision 19
# baseline (speedup 1.0000x reference)
"""Multi-head attention (dense_transformer) on 8 Trainium2 NeuronCores.

Reference computation (DIM=1024, HEADS=16, HEAD_DIM=64, SCALE=DIM**-0.5):
    qkv = x @ w_qkv                      # [b, n, 3*dim]
    q, k, v = split-heads(qkv)           # each [b, h, n, d]
    attn = softmax(q @ k^T * SCALE)
    out = (attn @ v) re-merged @ w_out + b_out

Sharding: 8 cores = (batch b in 0..3) x (head-group hg in 0..1, 8 heads each).
Each core computes a [2048, 1024] fp32 partial of the output projection for
its (batch, head-group); host sums the two head-group partials and adds bias.

Per-core dataflow (all matmuls fp16 operands, fp32 PSUM accumulate):
    qkT = wqk^T @ x^T      [1024, 2048]  (Q^T rows 0-511, K^T rows 512-1023)
    V   = x @ wv           [2048, 512]
    per (i-block of 512 queries, head pair):
        S^T[j, i] = K_h Q_h^T            (row-tiled pair: K=64 each)
        E = exp(S^T / 32)                (ScalarE, straight from PSUM)
        U^T[d, i] += V_h^T E             (col-tiled pair: M=64+64)
        r[i] = sum_j E[j, i]             (DVE chunk adds + ones-matmul reduce)
        U^T *= 1/r broadcast             (K=1 ones matmul broadcast)
    Y = U^T.T @ wout partial -> DRAM fp32
"""

import numpy as np

P = 128
DIM = 1024
NT = 2048          # tokens per batch
HL = 8             # heads per core (local)
HD = 64
KD = DIM // P      # 8 contraction chunks for the projections
NI = NT // 512     # 4 query blocks of 512
NJ = NT // P       # 16 key chunks of 128
SCALE = DIM ** -0.5

_CACHE = {}

def _build(loop_iters=None):
    from contextlib import ExitStack

    import concourse.bacc as bacc
    import concourse.tile as tile
    from concourse import mybir

    f16 = mybir.dt.float16
    f32 = mybir.dt.float32
    EXP = mybir.ActivationFunctionType.Exp

    nc = bacc.Bacc("TRN2", target_bir_lowering=False, debug=False)

    xT = nc.dram_tensor("xT", [DIM, NT], f32, kind="ExternalInput").ap()
    wqk = nc.dram_tensor("wqk", [DIM, 1024], f32, kind="ExternalInput").ap()
    wv = nc.dram_tensor("wv", [DIM, 512], f32, kind="ExternalInput").ap()
    wout = nc.dram_tensor("wout", [512, 1024], f32, kind="ExternalInput").ap()
    y = nc.dram_tensor("y", [NT, 1024], f32, kind="ExternalOutput").ap()

    with tile.TileContext(nc) as tc, ExitStack() as ctx, nc.allow_low_precision(
        reason="fp16 softmax-denominator accumulation, validated vs reference"
    ):
        persist = ctx.enter_context(tc.tile_pool(name="persist", bufs=1))
        stage = ctx.enter_context(tc.tile_pool(name="stage", bufs=2))
        epool = ctx.enter_context(tc.tile_pool(name="epool", bufs=6))
        rpool = ctx.enter_context(tc.tile_pool(name="rpool", bufs=4))
        ypool = ctx.enter_context(tc.tile_pool(name="ypool", bufs=3))
        ps_s = ctx.enter_context(tc.tile_pool(name="ps_s", bufs=2, space="PSUM"))
        ps_u = ctx.enter_context(tc.tile_pool(name="ps_u", bufs=2, space="PSUM"))
        ps_r = ctx.enter_context(tc.tile_pool(name="ps_r", bufs=2, space="PSUM"))

        xT_t = persist.tile([P, KD, NT], f16)        # x^T, fp16
        qkT_t = persist.tile([P, KD, NT], f16)       # [Q^T; K^T]
        V_t = persist.tile([P, NJ, 512], f16)        # V natural layout
        U_t = persist.tile([P, 4, NT], f16)          # U^T normalized, pair-chunked
        wqk_t = persist.tile([P, KD, 1024], f16)
        wv_t = persist.tile([P, KD, 512], f16)
        wout_t = persist.tile([P, 4, 1024], f16)
        ones_r = persist.tile([P, 1], f16)           # K=128, M=1 column-sum
        ones_b = persist.tile([1, 64], f16)          # K=1 broadcast
        nc.vector.memset(ones_r, 1.0)
        nc.vector.memset(ones_b, 1.0)

        def load_cast(dst, src_ap, cols):
            st = stage.tile([P, 2048], f32, tag="stage", name="st")
            nc.sync.dma_start(out=st[:, :cols], in_=src_ap)
            nc.gpsimd.tensor_copy(out=dst, in_=st[:, :cols])

        def qk_proj_chunk(m, n, pool, tag):
            ps = pool.tile([P, 512], f32, tag=tag, name="ps_qk")
            for k in range(KD):
                nc.tensor.matmul(
                    ps,
                    lhsT=wqk_t[:, k, m * P:(m + 1) * P],
                    rhs=xT_t[:, k, n * 512:(n + 1) * 512],
                    start=(k == 0), stop=(k == KD - 1),
                )
            nc.vector.tensor_copy(
                out=qkT_t[:, m, n * 512:(n + 1) * 512], in_=ps
            )

        def v_proj(mt, pool, tag):
            ps = pool.tile([P, 512], f32, tag=tag, name="ps_v")
            for k in range(KD):
                nc.tensor.matmul(
                    ps,
                    lhsT=xT_t[:, k, mt * P:(mt + 1) * P],
                    rhs=wv_t[:, k, :],
                    start=(k == 0), stop=(k == KD - 1),
                )
            nc.vector.tensor_copy(out=V_t[:, mt, :], in_=ps)

        def body(_iv=None):
            # ---- weight + x loads (fp32 DMA, cast to fp16) ----
            for k in range(KD):
                load_cast(wqk_t[:, k, :], wqk[k * P:(k + 1) * P, :], 1024)
            for k in range(KD):
                load_cast(wv_t[:, k, :], wv[k * P:(k + 1) * P, :], 512)
            for k in range(4):
                load_cast(wout_t[:, k, :], wout[k * P:(k + 1) * P, :], 1024)
            for k in range(KD):
                load_cast(xT_t[:, k, :], xT[k * P:(k + 1) * P, :], NT)

            # ---- projections needed before the pipeline starts ----
            for n in range(NI):
                qk_proj_chunk(0, n, ps_s, "s")
            for n in range(NI):
                qk_proj_chunk(4, n, ps_s, "s")
            for mt in range(8):
                v_proj(mt, ps_s, "s")

            # Remaining QKV work, interleaved into block-0 pipeline groups.
            # Pair p's qkT chunks must be fully emitted before its lookahead
            # S^T (2 groups before the pair starts); V chunk jc before the
            # PV at group jc of pair 0.
            extra_at = {}

            def add_extra(G, fn):
                extra_at.setdefault(G, []).append(fn)

            for jc in range(8, NJ):
                add_extra(jc - 8, lambda mt=jc: v_proj(mt, ps_r, "rr"))
            for pi, (ma, mb) in enumerate([(1, 5), (2, 6), (3, 7)]):
                g0 = pi * 16 + 2
                seq = [(ma, 0), (mb, 0), (ma, 1), (mb, 1),
                       (ma, 2), (mb, 2), (ma, 3), (mb, 3)]
                for t, (m, n) in enumerate(seq):
                    add_extra(
                        g0 + t,
                        lambda m=m, n=n: qk_proj_chunk(m, n, ps_r, "rr"),
                    )

            # ---- attention: flat software pipeline over (i, p, g) ----
            order = [
                (i, p, g) for i in range(NI) for p in range(4) for g in range(NJ)
            ]

            def st_group(i, p, g):
                """S^T for one j-chunk, both heads of the pair, row-tiled."""
                isl = slice(i * 512, (i + 1) * 512)
                jsl = slice(g * P, (g + 1) * P)
                s = ps_s.tile([P, 2, 512], f32, tag="s", name="s_ps")
                for hh in range(2):
                    pb = hh * 64
                    nc.tensor.matmul(
                        s[:, hh, :],
                        lhsT=qkT_t[pb:pb + 64, 4 + p, jsl],
                        rhs=qkT_t[pb:pb + 64, p, isl],
                        start=True, stop=True,
                        tile_position=(pb, 0),
                    )
                return s

            def pair_tail(i, p, u, racc):
                isl = slice(i * 512, (i + 1) * 512)
                rb = ps_r.tile([P, 512], f32, tag="rr", name="rb")
                for hh in range(2):
                    rp = ps_r.tile([P, 512], f32, tag="rr", name="rp")
                    nc.tensor.matmul(
                        rp[0:1, :], lhsT=ones_r, rhs=racc[:, hh, :],
                        start=True, stop=True,
                    )
                    rs = rpool.tile([1, 512], f16, tag=f"rs{hh}", name="rs")
                    nc.vector.reciprocal(out=rs, in_=rp[0:1, :])
                    nc.tensor.matmul(
                        rb[hh * 64:(hh + 1) * 64, :],
                        lhsT=ones_b, rhs=rs,
                        start=True, stop=True,
                        tile_position=(0, hh * 64),
                    )
                rb_sb = rpool.tile([P, 512], f16, tag="rb", name="rb_sb")
                nc.vector.tensor_copy(out=rb_sb, in_=rb)
                nc.vector.tensor_mul(out=U_t[:, p, isl], in0=u, in1=rb_sb)

            def out_proj(i):
                for m in range(4):
                    msl = slice(i * 512 + m * P, i * 512 + (m + 1) * P)
                    for n2 in range(2):
                        py = ps_r.tile([P, 512], f32, tag="rr", name="py")
                        for k in range(4):
                            nc.tensor.matmul(
                                py,
                                lhsT=U_t[:, k, msl],
                                rhs=wout_t[:, k, n2 * 512:(n2 + 1) * 512],
                                start=(k == 0), stop=(k == 3),
                            )
                        ysb = ypool.tile([P, 512], f32, tag="y", name="ysb")
                        nc.vector.tensor_copy(out=ysb, in_=py)
                        nc.sync.dma_start(
                            out=y[msl, n2 * 512:(n2 + 1) * 512], in_=ysb
                        )

            s_tiles = {0: st_group(*order[0]), 1: st_group(*order[1])}
            u = None
            racc = None
            for G, (i, p, g) in enumerate(order):
                s = s_tiles.pop(G)
                e = epool.tile([P, 2, 512], f16, tag="e", name="e")
                nc.scalar.activation(out=e[:], in_=s[:], func=EXP, scale=SCALE)
                if g == 0:
                    u = ps_u.tile([P, 512], f32, tag="u", name="u")
                    racc = rpool.tile([P, 2, 512], f16, tag="r", name="racc")
                for hh in range(2):
                    h = 2 * p + hh
                    nc.tensor.matmul(
                        u[hh * 64:(hh + 1) * 64, :],
                        lhsT=V_t[:, g, h * 64:(h + 1) * 64],
                        rhs=e[:, hh, :],
                        start=(g == 0), stop=(g == NJ - 1),
                        tile_position=(0, hh * 64),
                        skip_group_check=True,
                    )
                if G + 2 < len(order):
                    s_tiles[G + 2] = st_group(*order[G + 2])
                for fn in extra_at.get(G, ()):
                    fn()
                if g == 0:
                    nc.vector.tensor_copy(out=racc[:], in_=e[:])
                else:
                    nc.vector.tensor_add(racc[:], racc[:], e[:])
                if g == NJ - 1:
                    pair_tail(i, p, u, racc)
                    if p == 3:
                        out_proj(i)

        if loop_iters is None:
            body()
        else:
            with tc.For_i(0, loop_iters, 1) as iv:
                body(iv)

    nc.compile()
    return nc


def _in_maps(x, w_qkv, w_out):
    in_maps = []
    for bi in range(4):
        xTb = np.ascontiguousarray(x[bi].T)
        for hg in range(2):
            c = slice(hg * 512, (hg + 1) * 512)
            wqk = np.ascontiguousarray(
                np.concatenate([w_qkv[:, c], w_qkv[:, 1024:2048][:, c]], axis=1)
            )
            wv = np.ascontiguousarray(w_qkv[:, 2048:3072][:, c])
            wo = np.ascontiguousarray(w_out[c, :])
            in_maps.append({"xT": xTb, "wqk": wqk, "wv": wv, "wout": wo})
    return in_maps


def kernel(x, w_qkv, w_out, b_out):
    from concourse.bass_utils import run_bass_kernel_spmd

    if "nc" not in _CACHE:
        _CACHE["nc"] = _build()
    nc = _CACHE["nc"]

    x = np.ascontiguousarray(np.asarray(x, dtype=np.float32))
    w_qkv = np.asarray(w_qkv, dtype=np.float32)
    w_out = np.asarray(w_out, dtype=np.float32)
    b_out = np.asarray(b_out, dtype=np.float32)

    res = run_bass_kernel_spmd(
        nc, _in_maps(x, w_qkv, w_out), core_ids=list(range(8))
    )
    out = np.empty((4, NT, DIM), dtype=np.float32)
    for bi in range(4):
        out[bi] = res.results[2 * bi]["y"] + res.results[2 * bi + 1]["y"] + b_out
    return out


# revision 20
# speedup vs baseline: 2.1997x; 2.1997x over previous
"""Multi-head attention (dense_transformer) on 8 Trainium2 NeuronCores.

Reference computation (DIM=1024, HEADS=16, HEAD_DIM=64, SCALE=DIM**-0.5):
    qkv = x @ w_qkv                      # [b, n, 3*dim]
    q, k, v = split-heads(qkv)           # each [b, h, n, d]
    attn = softmax(q @ k^T * SCALE)
    out = (attn @ v) re-merged @ w_out + b_out

Sharding: 8 cores = (batch b in 0..3) x (head-group hg in 0..1, 8 heads each).
Each core computes a [2048, 1024] fp32 partial of the output projection for
its (batch, head-group); host sums the two head-group partials and adds bias.

Per-core dataflow (all matmuls fp16 operands, fp32 PSUM accumulate):
    qkT = wqk^T @ x^T      [1024, 2048]  (Q^T rows 0-511, K^T rows 512-1023)
    V   = x @ wv           [2048, 512]
    per (i-block of 512 queries, head pair):
        S^T[j, i] = K_h Q_h^T            (row-tiled pair: K=64 each)
        E = exp(S^T / 32)                (ScalarE, straight from PSUM)
        U^T[d, i] += V_h^T E             (col-tiled pair: M=64+64)
        r[i] = sum_j E[j, i]             (DVE chunk adds + ones-matmul reduce)
        U^T *= 1/r broadcast             (K=1 ones matmul broadcast)
    Y = U^T.T @ wout partial -> DRAM fp32
"""

import numpy as np

P = 128
DIM = 1024
NT = 2048          # tokens per batch
HL = 8             # heads per core (local)
HD = 64
KD = DIM // P      # 8 contraction chunks for the projections
NI = NT // 512     # 4 query blocks of 512
NJ = NT // P       # 16 key chunks of 128
SCALE = DIM ** -0.5

_CACHE = {}

def _build(loop_iters=None):
    from contextlib import ExitStack

    import concourse.bacc as bacc
    import concourse.tile as tile
    from concourse import mybir

    f16 = mybir.dt.float16
    f32 = mybir.dt.float32
    EXP = mybir.ActivationFunctionType.Exp

    nc = bacc.Bacc("TRN2", target_bir_lowering=False, debug=False)

    xT = nc.dram_tensor("xT", [DIM, NT], f32, kind="ExternalInput").ap()
    wqk = nc.dram_tensor("wqk", [DIM, 1024], f32, kind="ExternalInput").ap()
    wv = nc.dram_tensor("wv", [DIM, 512], f32, kind="ExternalInput").ap()
    wout = nc.dram_tensor("wout", [512, 1024], f32, kind="ExternalInput").ap()
    y = nc.dram_tensor("y", [NT, 1024], f32, kind="ExternalOutput").ap()

    with tile.TileContext(nc) as tc, ExitStack() as ctx, nc.allow_low_precision(
        reason="fp16 softmax-denominator accumulation, validated vs reference"
    ):
        persist = ctx.enter_context(tc.tile_pool(name="persist", bufs=1))
        stage = ctx.enter_context(tc.tile_pool(name="stage", bufs=2))
        epool = ctx.enter_context(tc.tile_pool(name="epool", bufs=8))
        rpool = ctx.enter_context(tc.tile_pool(name="rpool", bufs=4))
        ypool = ctx.enter_context(tc.tile_pool(name="ypool", bufs=3))
        ps_s = ctx.enter_context(tc.tile_pool(name="ps_s", bufs=2, space="PSUM"))
        ps_u = ctx.enter_context(tc.tile_pool(name="ps_u", bufs=2, space="PSUM"))
        ps_r = ctx.enter_context(tc.tile_pool(name="ps_r", bufs=2, space="PSUM"))

        xT_t = persist.tile([P, KD, NT], f16)        # x^T, fp16
        qkT_t = persist.tile([P, KD, NT], f16)       # [Q^T; K^T]
        V_t = persist.tile([P, NJ, 512], f16)        # V natural layout
        U_t = persist.tile([P, 4, NT], f16)          # U^T normalized, pair-chunked
        wqk_t = persist.tile([P, KD, 1024], f16)
        wv_t = persist.tile([P, KD, 512], f16)
        wout_t = persist.tile([P, 4, 1024], f16)
        ones_r = persist.tile([P, 1], f16)           # K=128, M=1 column-sum
        ones_b = persist.tile([1, 64], f16)          # K=1 broadcast
        nc.vector.memset(ones_r, 1.0)
        nc.vector.memset(ones_b, 1.0)

        def load_cast(dst, src_ap, cols):
            st = stage.tile([P, 2048], f32, tag="stage", name="st")
            nc.sync.dma_start(out=st[:, :cols], in_=src_ap)
            nc.gpsimd.tensor_copy(out=dst, in_=st[:, :cols])

        def qk_proj_chunk(m, n, pool, tag):
            ps = pool.tile([P, 512], f32, tag=tag, name="ps_qk")
            for k in range(KD):
                nc.tensor.matmul(
                    ps,
                    lhsT=wqk_t[:, k, m * P:(m + 1) * P],
                    rhs=xT_t[:, k, n * 512:(n + 1) * 512],
                    start=(k == 0), stop=(k == KD - 1),
                )
            nc.vector.tensor_copy(
                out=qkT_t[:, m, n * 512:(n + 1) * 512], in_=ps
            )

        def v_proj(mt, pool, tag):
            ps = pool.tile([P, 512], f32, tag=tag, name="ps_v")
            for k in range(KD):
                nc.tensor.matmul(
                    ps,
                    lhsT=xT_t[:, k, mt * P:(mt + 1) * P],
                    rhs=wv_t[:, k, :],
                    start=(k == 0), stop=(k == KD - 1),
                )
            nc.vector.tensor_copy(out=V_t[:, mt, :], in_=ps)

        def body(_iv=None):
            # ---- weight + x loads (fp32 DMA, cast to fp16) ----
            for k in range(KD):
                load_cast(wqk_t[:, k, :], wqk[k * P:(k + 1) * P, :], 1024)
            for k in range(KD):
                load_cast(wv_t[:, k, :], wv[k * P:(k + 1) * P, :], 512)
            for k in range(4):
                load_cast(wout_t[:, k, :], wout[k * P:(k + 1) * P, :], 1024)
            for k in range(KD):
                load_cast(xT_t[:, k, :], xT[k * P:(k + 1) * P, :], NT)

            # ---- projections needed before the pipeline starts ----
            for n in range(NI):
                qk_proj_chunk(0, n, ps_s, "s")
            for n in range(NI):
                qk_proj_chunk(4, n, ps_s, "s")
            for mt in range(8):
                v_proj(mt, ps_s, "s")

            # Remaining QKV work, interleaved into block-0 pipeline groups.
            # Pair p's qkT chunks must be fully emitted before its lookahead
            # S^T (2 groups before the pair starts); V chunk jc before the
            # PV at group jc of pair 0.
            extra_at = {}

            def add_extra(G, fn):
                extra_at.setdefault(G, []).append(fn)

            for jc in range(8, NJ):
                add_extra(jc - 8, lambda mt=jc: v_proj(mt, ps_r, "rr"))
            for pi, (ma, mb) in enumerate([(1, 5), (2, 6), (3, 7)]):
                g0 = pi * 16 + 2
                seq = [(ma, 0), (mb, 0), (ma, 1), (mb, 1),
                       (ma, 2), (mb, 2), (ma, 3), (mb, 3)]
                for t, (m, n) in enumerate(seq):
                    add_extra(
                        g0 + t,
                        lambda m=m, n=n: qk_proj_chunk(m, n, ps_r, "rr"),
                    )

            # ---- attention: flat software pipeline over (i, p, g) ----
            order = [
                (i, p, g) for i in range(NI) for p in range(4) for g in range(NJ)
            ]

            def st_group(i, p, g):
                """S^T for one j-chunk, both heads of the pair, row-tiled."""
                isl = slice(i * 512, (i + 1) * 512)
                jsl = slice(g * P, (g + 1) * P)
                s = ps_s.tile([P, 2, 512], f32, tag="s", name="s_ps")
                for hh in range(2):
                    pb = hh * 64
                    nc.tensor.matmul(
                        s[:, hh, :],
                        lhsT=qkT_t[pb:pb + 64, 4 + p, jsl],
                        rhs=qkT_t[pb:pb + 64, p, isl],
                        start=True, stop=True,
                        tile_position=(pb, 0),
                    )
                return s

            def pair_tail(i, p, u, racc):
                isl = slice(i * 512, (i + 1) * 512)
                rb = ps_r.tile([P, 512], f32, tag="rr", name="rb")
                for hh in range(2):
                    rp = ps_r.tile([P, 512], f32, tag="rr", name="rp")
                    nc.tensor.matmul(
                        rp[0:1, :], lhsT=ones_r, rhs=racc[:, hh, :],
                        start=True, stop=True,
                    )
                    rs = rpool.tile([1, 512], f16, tag=f"rs{hh}", name="rs")
                    nc.vector.reciprocal(out=rs, in_=rp[0:1, :])
                    nc.tensor.matmul(
                        rb[hh * 64:(hh + 1) * 64, :],
                        lhsT=ones_b, rhs=rs,
                        start=True, stop=True,
                        tile_position=(0, hh * 64),
                    )
                rb_sb = rpool.tile([P, 512], f16, tag="rb", name="rb_sb")
                nc.vector.tensor_copy(out=rb_sb, in_=rb)
                nc.vector.tensor_mul(out=U_t[:, p, isl], in0=u, in1=rb_sb)

            def out_proj_chunk(i, m, n2):
                msl = slice(i * 512 + m * P, i * 512 + (m + 1) * P)
                py = ps_r.tile([P, 512], f32, tag="rr", name="py")
                for k in range(4):
                    nc.tensor.matmul(
                        py,
                        lhsT=U_t[:, k, msl],
                        rhs=wout_t[:, k, n2 * 512:(n2 + 1) * 512],
                        start=(k == 0), stop=(k == 3),
                    )
                ysb = ypool.tile([P, 512], f32, tag="y", name="ysb")
                nc.vector.tensor_copy(out=ysb, in_=py)
                nc.sync.dma_start(
                    out=y[msl, n2 * 512:(n2 + 1) * 512], in_=ysb
                )

            late_at = {}
            for i in range(NI - 1):
                for t, (m, n2) in enumerate(
                    [(m, n2) for m in range(4) for n2 in range(2)]
                ):
                    # block i's out-proj runs during block i+1, one chunk
                    # every other group
                    late_at.setdefault((i + 1) * 64 + 2 * t + 1, []).append(
                        lambda i=i, m=m, n2=n2: out_proj_chunk(i, m, n2)
                    )

            s_tiles = {0: st_group(*order[0]), 1: st_group(*order[1])}
            u = None
            racc = None
            for G, (i, p, g) in enumerate(order):
                s = s_tiles.pop(G)
                e = epool.tile([P, 2, 512], f16, tag="e", name="e")
                nc.scalar.activation(out=e[:], in_=s[:], func=EXP, scale=SCALE)
                if g == 0:
                    u = ps_u.tile([P, 512], f32, tag="u", name="u")
                    racc = rpool.tile([P, 2, 512], f16, tag="r", name="racc")
                for hh in range(2):
                    h = 2 * p + hh
                    nc.tensor.matmul(
                        u[hh * 64:(hh + 1) * 64, :],
                        lhsT=V_t[:, g, h * 64:(h + 1) * 64],
                        rhs=e[:, hh, :],
                        start=(g == 0), stop=(g == NJ - 1),
                        tile_position=(0, hh * 64),
                        skip_group_check=True,
                    )
                if G + 2 < len(order):
                    s_tiles[G + 2] = st_group(*order[G + 2])
                for fn in extra_at.get(G, ()):
                    fn()
                if g == 0:
                    nc.vector.tensor_copy(out=racc[:], in_=e[:])
                else:
                    nc.vector.tensor_add(racc[:], racc[:], e[:])
                if g == NJ - 1:
                    pair_tail(i, p, u, racc)
                for fn in late_at.get(G, ()):
                    fn()
            for m in range(4):
                for n2 in range(2):
                    out_proj_chunk(NI - 1, m, n2)

        if loop_iters is None:
            body()
        else:
            with tc.For_i(0, loop_iters, 1) as iv:
                body(iv)

    nc.compile()
    return nc


def _in_maps(x, w_qkv, w_out):
    in_maps = []
    for bi in range(4):
        xTb = np.ascontiguousarray(x[bi].T)
        for hg in range(2):
            c = slice(hg * 512, (hg + 1) * 512)
            wqk = np.ascontiguousarray(
                np.concatenate([w_qkv[:, c], w_qkv[:, 1024:2048][:, c]], axis=1)
            )
            wv = np.ascontiguousarray(w_qkv[:, 2048:3072][:, c])
            wo = np.ascontiguousarray(w_out[c, :])
            in_maps.append({"xT": xTb, "wqk": wqk, "wv": wv, "wout": wo})
    return in_maps


def kernel(x, w_qkv, w_out, b_out):
    from concourse.bass_utils import run_bass_kernel_spmd

    if "nc" not in _CACHE:
        _CACHE["nc"] = _build()
    nc = _CACHE["nc"]

    x = np.ascontiguousarray(np.asarray(x, dtype=np.float32))
    w_qkv = np.asarray(w_qkv, dtype=np.float32)
    w_out = np.asarray(w_out, dtype=np.float32)
    b_out = np.asarray(b_out, dtype=np.float32)

    res = run_bass_kernel_spmd(
        nc, _in_maps(x, w_qkv, w_out), core_ids=list(range(8))
    )
    out = np.empty((4, NT, DIM), dtype=np.float32)
    for bi in range(4):
        out[bi] = res.results[2 * bi]["y"] + res.results[2 * bi + 1]["y"] + b_out
    return out
